# revision 8
# baseline (speedup 1.0000x reference)
"""Trainium2 Bass kernel for ChromophoreSolventTransformerGNN.

Strategy: dst-node sharding across 8 cores (4096 nodes / 32 blocks of 128 each).
Edges sorted by dst, padded per 128-node block to LPAD slots. Segment softmax
and segment sum become dense selector matmuls (one-hot S matrices built on
host from the index structure). All heavy matmuls run as float32r (full-rate
PE). Layer-1 output h1 is exchanged via AllGather; pooling partials are
combined with dma_scatter_add + ReduceScatter; the MLP head runs per-core on
its 128 graphs.
"""
import numpy as np
import ml_dtypes
_bf16 = ml_dtypes.bfloat16

N, E, B, H = 32768, 131072, 1024, 4
F_NODE, F_EDGE, F_SOLV = 64, 16, 128
CH1, CH2 = 128, 256
D1, D2 = CH1 * H, CH2 * H          # 512, 1024
NCORES = 8
NPC = N // NCORES                   # nodes per core = 4096
NBLK = NPC // 128                   # blocks per core = 32
BN_EPS = 1e-5
BIG = 200.0


# ----------------------------------------------------------------------------
# host-side index prep
# ----------------------------------------------------------------------------

def _pack_idx16(idx, pad_cols=None):
    """Pack indices: slot i -> [i%16, i//16]; replicate 16 rows to 128."""
    n = len(idx)
    cols = (n + 15) // 16 if pad_cols is None else pad_cols
    a = np.zeros((16, cols), dtype=np.int16)
    a[np.arange(n) % 16, np.arange(n) // 16] = idx.astype(np.int16)
    return np.tile(a, (8, 1))


def host_prep(x, edge_index, edge_attr, batch_ids, solvent, params):
    x = np.asarray(x, np.float32)
    edge_index = np.asarray(edge_index, np.int64)
    edge_attr = np.asarray(edge_attr, np.float32)
    batch_ids = np.asarray(batch_ids, np.int64)
    solvent = np.asarray(solvent, np.float32)
    src, dst = edge_index[0], edge_index[1]

    order = np.argsort(dst, kind="stable")
    dst_s, src_s = dst[order], src[order]
    blk_of_edge = dst_s // 128
    blk_cnt = np.bincount(blk_of_edge, minlength=N // 128)
    LPAD = int(-(-blk_cnt.max() // 128) * 128)
    G = LPAD // 128
    blk_start = np.concatenate([[0], np.cumsum(blk_cnt)])

    nblk_total = N // 128
    # slot table: edge id per (block, slot), -1 for pads
    slot_edge = np.full((nblk_total, LPAD), -1, np.int64)
    for bk in range(nblk_total):
        cnt = blk_cnt[bk]
        slot_edge[bk, :cnt] = order[blk_start[bk]:blk_start[bk] + cnt]

    valid = slot_edge >= 0
    se = np.where(valid, slot_edge, 0)
    src_pad = np.where(valid, src[se], 0)                       # [nblk, LPAD]
    dstloc_pad = np.where(valid, dst[se] - (np.arange(nblk_total) * 128)[:, None], -1)

    # S one-hot node-major [nblk, 128, LPAD]
    S = np.zeros((nblk_total, 128, LPAD), np.float32)
    bidx, sidx = np.nonzero(valid)
    S[bidx, dstloc_pad[bidx, sidx], sidx] = 1.0

    # eaT augmented [nblk, 17, LPAD]
    eaT = np.zeros((nblk_total, 17, LPAD), np.float32)
    ea_pad = np.where(valid[:, :, None], edge_attr[se], 0.0)     # [nblk, LPAD, 16]
    eaT[:, :16, :] = ea_pad.transpose(0, 2, 1)
    eaT[:, 16, :] = 1.0

    # per-core packed gather indices [NBLK, 128, LPAD//16]
    src16 = np.zeros((nblk_total, 128, LPAD // 16), np.int16)
    for bk in range(nblk_total):
        src16[bk] = _pack_idx16(src_pad[bk])
    # L2 gathers from the rank-major half-gathered h1 buffer:
    # halfA of core c lands at rows [c*2048, (c+1)*2048); halfB at 16384 + same
    nn = src_pad
    off = (nn // NPC) * 2048 + (nn % NPC) % 2048
    src_r = np.where((nn % NPC) < 2048, off, N // 2 + off)
    src16r = np.zeros((nblk_total, 128, LPAD // 16), np.int16)
    for bk in range(nblk_total):
        src16r[bk] = _pack_idx16(src_r[bk])

    # pooling: per core, per half (2048 nodes), local slot = graph - g_lo
    P = np.zeros((nblk_total, 128, 128), np.float32)
    sidx16 = np.zeros((NCORES, 2, 128, 8), np.int16)
    g_of_node = batch_ids
    for c in range(NCORES):
        for hh in range(2):
            lo_node = c * NPC + hh * 2048
            g_lo = int(g_of_node[lo_node])
            g_hi = int(g_of_node[lo_node + 2047])
            span = g_hi - g_lo + 1
            assert span <= 128, f"graph span {span} > 128"
            # pads carry zero rows; give them distinct targets to avoid
            # same-row RMW races in the scatter-add
            rows = np.arange(g_lo, g_lo + 128) % B
            sidx16[c, hh] = _pack_idx16(rows, pad_cols=8)
            for bb in range(16):
                bk = c * NBLK + hh * 16 + bb
                gl = g_of_node[bk * 128:(bk + 1) * 128] - g_lo
                P[bk, np.arange(128), gl] = 1.0

    cnt = np.bincount(batch_ids, minlength=B).astype(np.float32)
    recip_cnt = 1.0 / np.maximum(cnt, 1.0)

    # ---- weights ----
    pr = {k: {kk: (np.asarray(vv, np.float32) if vv is not None else None)
              for kk, vv in v.items()} for k, v in params.items()}

    def bn_fold(p, bskip):
        a = p["gamma"] / np.sqrt(p["var"] + BN_EPS)
        b = p["beta"] - p["mean"] * a
        bp = b + a * bskip
        return a.astype(np.float32), bp.astype(np.float32)

    a1, b1p = bn_fold(pr["bn1"], pr["conv1"]["bskip"])
    a2, b2p = bn_fold(pr["bn2"], pr["conv2"]["bskip"])

    def aug(We, bias, d):
        w = np.zeros((17, d), np.float32)
        w[:16] = We
        w[16] = bias
        return w

    c1, c2 = pr["conv1"], pr["conv2"]
    WeK1 = aug(c1["We"], c1["bk"], D1)
    WeV1 = aug(c1["We"], c1["bv"], D1) * a1[None, :]
    WeK2 = aug(c2["We"], c2["bk"], D2)
    WeV2 = aug(c2["We"], c2["bv"], D2) * a2[None, :]

    afc = pr["bn_fc1"]["gamma"] / np.sqrt(pr["bn_fc1"]["var"] + BN_EPS)
    bfc = (afc * (pr["fc1"]["b"] - pr["bn_fc1"]["mean"]) + pr["bn_fc1"]["beta"]).astype(np.float32)

    W1 = pr["fc1"]["W"]                          # [1152, 128]
    W1c = W1.reshape(9, 128, 128)                # k-chunks

    shared = dict(
        x_tab=x,                                                   # [N, 64]
        ident=np.eye(128, dtype=np.float32),
        identbf=np.eye(128, dtype=np.float32).astype(_bf16),
        bigi1=(BIG * np.sqrt(CH1)) * np.eye(128, dtype=np.float32),
        bigi2=(BIG * np.sqrt(CH2)) * np.eye(128, dtype=np.float32),
        wq1=c1["Wq"], wk1=c1["Wk"], wv1=c1["Wv"] * a1[None, :], wsk1=c1["Wskip"] * a1[None, :],
        wek1=WeK1, wev1=WeV1,
        bq1=c1["bq"].reshape(4, 128).T.copy(),                     # [128, 4]
        bk1=c1["bk"].reshape(4, 128).T.copy(),
        wq2=c2["Wq"], wsk2=c2["Wskip"] * a2[None, :],
        wk2bf=c2["Wk"].astype(_bf16), wv2bf=(c2["Wv"] * a2[None, :]).astype(_bf16),
        wek2=WeK2, wev2=WeV2,
        bq2=c2["bq"].reshape(8, 128).T.copy(),                     # [128, 8]
        bk2=c2["bk"].reshape(8, 128).T.copy(),
        a1rep=np.tile(a1, (128, 1)), b1prep=np.tile(b1p, (128, 1)),
        a2rep=np.tile(a2, (128, 1)), b2prep=np.tile(b2p, (128, 1)),
        w1c=W1c.transpose(1, 0, 2).copy(),                         # [128, 9, 128]
        ws=pr["fc_solvent"]["W"],                                  # [128, 128]
        bs=pr["fc_solvent"]["b"].reshape(128, 1),
        w2=pr["fc2"]["W"].reshape(128, 1),
        b2=pr["fc2"]["b"].reshape(1, 1),
        fcscale=afc.reshape(128, 1).astype(np.float32),
        fcbias=bfc.reshape(128, 1),
    )

    in_maps = []
    for c in range(NCORES):
        sl = slice(c * NBLK, (c + 1) * NBLK)
        g0 = c * 128
        m = dict(shared)
        m.update(
            S=S[sl], eaT=eaT[sl], src16=src16[sl], src16r=src16r[sl], P=P[sl],
            sidx16=sidx16[c].reshape(2 * 128, 8),
            xT_own=x[c * NPC:(c + 1) * NPC].T.copy(),              # [64, 4096]
            solvT_own=solvent[g0:g0 + 128].T.copy(),               # [128, 128]
            reciprep=np.tile(recip_cnt[g0:g0 + 128], (128, 1)),    # [128, 128]
        )
        in_maps.append({k: np.ascontiguousarray(v) for k, v in m.items()})
    return in_maps, LPAD


# ----------------------------------------------------------------------------
# numpy simulator of the exact device dataflow (for validation)
# ----------------------------------------------------------------------------

def numpy_sim(x, edge_index, edge_attr, batch_ids, solvent, params):
    in_maps, LPAD = host_prep(x, edge_index, edge_attr, batch_ids, solvent, params)
    G = LPAD // 128
    x = np.asarray(x, np.float32)

    def unpack16(a16, n):
        a = a16[:16]
        return np.array([a[i % 16, i // 16] for i in range(n)], np.int64)

    pool_global = np.zeros((B, D2), np.float32)
    out_all = np.zeros((B, 1), np.float32)
    h1_own_all = {}

    for c in range(NCORES):
        m = in_maps[c]
        # ---- layer 1 ----
        h1_own = np.zeros((NPC, D1), np.float32)
        for b in range(NBLK):
            idx = unpack16(m["src16"][b].astype(np.int64), LPAD)
            xs = x[idx]                                  # [LPAD, 64]
            ke = xs @ m["wk1"] + m["eaT"][b].T @ m["wek1"]      # [LPAD, 512]
            ve = xs @ m["wv1"] + m["eaT"][b].T @ m["wev1"]
            q = m["xT_own"][:, b * 128:(b + 1) * 128].T @ m["wq1"] + np.concatenate([m["bq1"][:, i] for i in range(4)])
            S = m["S"][b]                                # [128, LPAD]
            expv = np.zeros((4, 128, LPAD), np.float32)
            denom = np.zeros((128, 4), np.float32)
            for h in range(4):
                sc = q[:, h * 128:(h + 1) * 128] @ ke[:, h * 128:(h + 1) * 128].T  # [128, LPAD]
                sc = sc + (BIG * np.sqrt(CH1)) * S
                ev = np.exp(sc / np.sqrt(CH1) - BIG).astype(np.float32)
                expv[h] = ev
                denom[:, h] = ev.sum(1)
            agg = np.zeros((128, D1), np.float32)
            for h in range(4):
                agg[:, h * 128:(h + 1) * 128] = expv[h] @ ve[:, h * 128:(h + 1) * 128]
            recip = 1.0 / (denom + 1e-30)
            skip = m["xT_own"][:, b * 128:(b + 1) * 128].T @ m["wsk1"]
            hpre = agg * np.repeat(recip, 128, axis=1) + skip
            hblk = np.maximum(hpre + m["b1prep"][0], 0)
            h1_own[b * 128:(b + 1) * 128] = hblk
        h1_own_all[c] = h1_own

    h1_full = np.concatenate([h1_own_all[c] for c in range(NCORES)], 0)

    for c in range(NCORES):
        m = in_maps[c]
        pool_half = np.zeros((2, 128, D2), np.float32)
        for b in range(NBLK):
            idx = unpack16(m["src16"][b].astype(np.int64), LPAD)
            hs = h1_full[idx]
            ke = hs @ m["wk2bf"].astype(np.float32) + m["eaT"][b].T @ m["wek2"]
            ve = hs @ m["wv2bf"].astype(np.float32) + m["eaT"][b].T @ m["wev2"]
            hT_own = h1_full[c * NPC + b * 128: c * NPC + (b + 1) * 128]
            q = hT_own @ m["wq2"] + np.concatenate([m["bq2"][:, i] for i in range(8)])
            S = m["S"][b]
            expv = np.zeros((4, 128, LPAD), np.float32)
            denom = np.zeros((128, 4), np.float32)
            for h in range(4):
                sc = q[:, h * 256:(h + 1) * 256] @ ke[:, h * 256:(h + 1) * 256].T
                sc = sc + (BIG * np.sqrt(CH2)) * S
                ev = np.exp(sc / np.sqrt(CH2) - BIG).astype(np.float32)
                expv[h] = ev
                denom[:, h] = ev.sum(1)
            agg = np.zeros((128, D2), np.float32)
            for h in range(4):
                agg[:, h * 256:(h + 1) * 256] = expv[h] @ ve[:, h * 256:(h + 1) * 256]
            recip = 1.0 / (denom + 1e-30)
            skip = hT_own @ m["wsk2"]
            hpre = agg * np.repeat(recip, 256, axis=1) + skip
            h2 = np.maximum(hpre + m["b2prep"][0], 0)
            pool_half[b // 16] += m["P"][b].T @ h2
        # scatter
        for hh in range(2):
            rows = unpack16(m["sidx16"].reshape(2, 128, 8)[hh].astype(np.int64), 128)
            for s in range(128):
                pool_global[rows[s]] += pool_half[hh, s]

    # tail per core
    for c in range(NCORES):
        m = in_maps[c]
        g_own = pool_global[c * 128:(c + 1) * 128]           # RS result
        zT = (g_own * m["reciprep"][0][:, None]).T           # [1024, 128]
        sT = np.maximum(m["ws"].T @ m["solvT_own"] + m["bs"], 0)   # [128, 128]
        z1 = np.zeros((128, 128), np.float32)
        for kc in range(8):
            z1 += m["w1c"][:, kc, :].T @ zT[kc * 128:(kc + 1) * 128]
        z1 += m["w1c"][:, 8, :].T @ sT
        z1 = np.maximum(m["fcscale"] * z1 + m["fcbias"], 0)  # [128 f, 128 g]
        o = m["w2"].T @ z1 + m["b2"]                         # [1, 128]
        out_all[c * 128:(c + 1) * 128, 0] = o[0]
    return out_all


if __name__ == "__main__":
    import pickle
    d = np.load("/root/problem/inputs_cache.npz")
    with open("/root/problem/params_cache.pkl", "rb") as f:
        params = pickle.load(f)
    ref = np.load("/root/problem/ref_out.npy")
    out = numpy_sim(d["x"], d["edge_index"], d["edge_attr"], d["batch_ids"],
                    d["solvent_fingerprint"], params)
    err = np.abs(out - ref).max()
    rel = err / np.abs(ref).max()
    print("numpy_sim vs reference: max abs err", err, "rel", rel)
